# revision 30
# baseline (speedup 1.0000x reference)
"""Causal self-attention for TRN2, 8 NeuronCores.

Problem (hardcoded): B=4, T=2048, C=1024, H=16 heads, hd=64.
  qkv = x @ qkv_w.T + qkv_b ; per-head causal softmax(q k^T / 8) @ v ; out @ proj_w.T + proj_b

Sharding: core c -> (batch b = c//2, head-half g = c%2). Each core owns one
batch and 8 heads (512 q/k/v channels), so x traffic is 1/4 of the
all-batches variant and the proj partial is [1024, 2048] summed host-side
over just 2 cores per batch.

qkv projection runs in compensated fp8 (e4m3) with DoubleRow matmuls:
host pre-splits x and 64*w into (hi, lo) fp8 pairs and the device computes
xh@wh + xh@wl + xl@wh, each a 2-subtile (256-contraction) DR matmul at
0.5 cycles/row -- 3/4 the PE cycles of fp32r at ~1e-3 total error.
All scale factors are absorbed algebraically: q,k,v live as 64*(q,k,v) in
bf16, the exp scale is 0.125/64^2 = 2^-15, and the softmax-sum ones column
is 64.0 so the v scale cancels in the reciprocal-normalize.

v is produced directly token-major (stationary = x chunk, moving = wv), so
no PE transposes are needed for the attention v-matmul.

Attention per head-pair hp (2 heads packed on 128 partitions):
  scoresT[kt, q] per 128-token k-chunk via bf16 matmuls (+ additive -1e9
  causal mask on the diagonal 128 block), exp via ACT (psum -> bf16 sbuf),
  poT[d, q] = [64*v | 64].T @ ex accumulated in PSUM (row 64 = 64*softmax
  sums), normalization via DVE reciprocal + gpsimd partition-broadcast +
  DVE multiply into a bf16 attn slab.

proj: yT[o, t] = pw.T @ attn contracting this core's 512 channels over 4
slab matmuls; Pool evicts psum -> sbuf; host sums 2 partials per batch.

Engine budget per core (cost model): PE ~508K cycles (~212us), ACT ~157us
(exp), DVE ~60us, Pool ~90us, DMA ~17MB (~50us). Software pipelining:
phase A does qkv for v(all heads) + q/k(hp0); each attention stage hp
interleaves q/k units of hp+1; hp3 interleaves proj units per finished
512-token block.
"""

import numpy as np
import ml_dtypes

import concourse.mybir as mybir
import concourse.tile as tile
from concourse import bacc
from concourse.bass_utils import run_bass_kernel_spmd
from concourse.masks import make_identity

F32 = mybir.dt.float32
F32R = mybir.dt.float32r
BF16 = mybir.dt.bfloat16
F8 = mybir.dt.float8e4
ACT_F = mybir.ActivationFunctionType
ALU = mybir.AluOpType
DR = mybir.MatmulPerfMode.DoubleRow
E4M3 = ml_dtypes.float8_e4m3

B, T, C, H, HD = 4, 2048, 1024, 16, 64
NCORES = 8
P = 128
GC = 512            # channels per core (8 heads)
NHP = 4             # head-pairs per core
NEG = -1.0e12
SX = 16.0           # host-side x prescale (keeps fp8 lo-terms normal-range)
SW = 512.0          # host-side weight prescale
SQK = SX * SW       # scale carried by q,k,v in sbuf (8192)
ONES = SW           # softmax-sum column: denom scale 512 -> attn slab = 16*attn
EXP_SCALE = 0.125 / (SQK * SQK)   # exact 2^-29

_CACHED = {}


def _round_fp32r(a: np.ndarray) -> np.ndarray:
    u = np.ascontiguousarray(a, dtype=np.float32).view(np.uint32)
    lsb = (u >> 12) & 1
    out = ((u + 0x7FF + lsb) & np.uint32(0xFFFFF000)).view(np.float32)
    return np.where(np.isfinite(a), out, a).astype(np.float32)


class _Ctx:
    pass


def _build():
    nc = bacc.Bacc("TRN2", target_bir_lowering=False, debug=False)

    xh_d = nc.dram_tensor("xh", [C, T], F8, kind="ExternalInput").ap()
    xl_d = nc.dram_tensor("xl", [C, T], F8, kind="ExternalInput").ap()
    w_ds = {}
    for name in ("wqh", "wql", "wkh", "wkl", "wvh", "wvl"):
        w_ds[name] = nc.dram_tensor(name, [C, GC], F8, kind="ExternalInput").ap()
    pwh_d = nc.dram_tensor("pwh", [GC, C], F8, kind="ExternalInput").ap()
    pwl_d = nc.dram_tensor("pwl", [GC, C], F8, kind="ExternalInput").ap()
    qb_d = nc.dram_tensor("qb64", [P, NHP], F32, kind="ExternalInput").ap()
    kb_d = nc.dram_tensor("kb64", [P, NHP], F32, kind="ExternalInput").ap()
    vb_d = nc.dram_tensor("vb64", [1, GC], F32, kind="ExternalInput").ap()
    yT_d = nc.dram_tensor("yT", [C, T], F32, kind="ExternalOutput").ap()
    import os
    DBG = os.environ.get("KDBG", "0") == "1"
    if DBG:
        qT_dbg = nc.dram_tensor("qT_dbg", [P, NHP * T], F32, kind="ExternalOutput").ap()
        kT_dbg = nc.dram_tensor("kT_dbg", [P, NHP * T], F32, kind="ExternalOutput").ap()
        v_dbg = nc.dram_tensor("v_dbg", [P, 16 * 8 * 65], BF16, kind="ExternalOutput").ap()
        at_dbg = nc.dram_tensor("at_dbg", [P, NHP * T], BF16, kind="ExternalOutput").ap()
        a8h_dbg = nc.dram_tensor("a8h_dbg", [P, NHP * T], F8, kind="ExternalOutput").ap()
        a8l_dbg = nc.dram_tensor("a8l_dbg", [P, NHP * T], F8, kind="ExternalOutput").ap()

    with tile.TileContext(nc) as tc:
        with (
            tc.tile_pool(name="const", bufs=1) as pc,
            tc.tile_pool(name="exp", bufs=4) as pe,
            tc.tile_pool(name="small", bufs=2) as psm,
            tc.tile_pool(name="yev", bufs=4) as py,
            tc.tile_pool(name="ps", bufs=1, space="PSUM") as pp,
        ):
            g = _Ctx()
            g.nc = nc

            # PE warmup: all-zero matmuls bridge the initial DMA latency so
            # the tensor engine enters the real stream already ramped (the
            # cost model halves matmul speed until 3us of continuous busy)
            zed = pc.tile([P, 512], BF16, tag="zed")
            nc.vector.memset(zed[:], 0.0)
            for wi in range(20):
                dps = pp.tile([P, 512], F32, tag="acc", bufs=2,
                              name=f"warm_{wi}")
                nc.tensor.matmul(dps[:], zed[:, 0:P], zed[:],
                                 start=True, stop=True)

            # ---- constants ----
            identb = pc.tile([P, P], BF16, tag="identb")
            make_identity(nc, identb)
            maskb = pc.tile([P, P], BF16, tag="maskb")
            nc.gpsimd.memset(maskb[:], 0.0)
            nc.gpsimd.affine_select(
                out=maskb[:], in_=maskb[:],
                compare_op=ALU.is_ge, fill=NEG, base=0,
                pattern=[[1, P]], channel_multiplier=-1,
            )

            # ---- weights / biases ----
            # first-compute-critical DMAs first, split per-kp so the first
            # matmul only waits for one kp slice of wvh + x0h
            wt = {}
            for name in ("wvh", "wvl", "wqh", "wql", "wkh", "wkl"):
                t = pc.tile([P, 4, 2, GC], F8, tag=name, name=f"w_{name}")
                wt[name] = t
            xh_sb = pc.tile([P, 4, 2, T], F8, tag="xh")
            xl_sb = pc.tile([P, 4, 2, T], F8, tag="xl")

            def load_x(tc_i, hl="hl", ntc=1):
                pairs = {"h": (xh_sb, xh_d), "l": (xl_sb, xl_d)}
                for key in hl:
                    sb, d = pairs[key]
                    nc.sync.dma_start(
                        sb[:, :, :, tc_i * 512:(tc_i + ntc) * 512],
                        d.rearrange("(kp s p) t -> p kp s t", p=P, s=2)
                        [:, :, :, tc_i * 512:(tc_i + ntc) * 512],
                    )

            def load_w(name):
                nc.sync.dma_start(
                    wt[name][:],
                    w_ds[name].rearrange("(kp s p) c -> p kp s c", p=P, s=2),
                )

            # DMA queue ordered by first use, whole-tensor transfers (the
            # HWDGE descriptor-gen costs ~625ns per DMA, so fine splitting
            # paces PE at the HWDGE rate instead of the transfer rate)
            load_w("wvh")
            load_x(0, "h")
            vbrow = pc.tile([1, GC], F32, tag="vbrow")
            nc.sync.dma_start(vbrow[:], vb_d[:])
            load_x(0, "l")
            load_w("wvl")
            qb = pc.tile([P, NHP], F32, tag="qb")
            nc.sync.dma_start(qb[:], qb_d[:])
            kb = pc.tile([P, NHP], F32, tag="kb")
            nc.sync.dma_start(kb[:], kb_d[:])
            load_w("wqh")
            load_w("wql")
            load_w("wkh")
            load_w("wkl")
            load_x(1)
            vbb = pc.tile([P, GC], F32, tag="vbb")
            nc.gpsimd.partition_broadcast(vbb[:], vbrow[0:1, :])

            pw8h = pc.tile([P, 2, 2, C], F8, tag="pw8h")
            pw8l = pc.tile([P, 2, 2, C], F8, tag="pw8l")
            g.pw_loaded = False

            def ensure_pw():
                if not g.pw_loaded:
                    for t, d in ((pw8h, pwh_d), (pw8l, pwl_d)):
                        nc.sync.dma_start(
                            t[:],
                            d.rearrange("(kp s p) o -> p kp s o", p=P, s=2))
                    g.pw_loaded = True

            # ---- big state ----
            # qT/kT/attn are F32R: f32-family moving operands avoid the
            # per-matmul InstLdweights (71ns PE SEQ each) that non-f32
            # ifmaps require; ex/v_sb stay BF16 (SEQ has headroom there).
            qT = pc.tile([P, NHP, T], F32R, tag="qT")     # 64*q, part = l*64+d
            kT = pc.tile([P, NHP, T], F32R, tag="kT")     # 64*k
            v_sb = pc.tile([P, 16, 8, 65], BF16, tag="v_sb")  # [tok, ck, h, d|one]
            attn = pc.tile([P, NHP, T], BF16, tag="attn")
            attn8h = pc.tile([P, 2, 2, T], F8, tag="attn8h")
            attn8l = pc.tile([P, 2, 2, T], F8, tag="attn8l")
            # softmax-sum column: 64.0 so the 64*v scale cancels at normalize
            nc.vector.memset(v_sb[:, :, :, 64:65], ONES)

            # ---- qkv units (fp8 DoubleRow, 3-term compensated) ----
            def qkv_qk_unit(which, hp, tc_i):
                """q or k for head-pair hp over 512 tokens: psum [128ch, 512]."""
                wh, wl = ("wqh", "wql") if which == "q" else ("wkh", "wkl")
                dst, bias = (qT, qb) if which == "q" else (kT, kb)
                acc = pp.tile([P, 512], F32, tag="acc", bufs=2,
                              name=f"acc_{which}_{hp}_{tc_i}")
                terms = [(xh_sb, wt[wh]), (xl_sb, wt[wh]), (xh_sb, wt[wl])]
                n = len(terms) * 4
                i = 0
                for xs, ws in terms:
                    for kp in range(4):
                        nc.tensor.matmul(
                            acc[:],
                            ws[:, kp, :, hp * P:(hp + 1) * P],
                            xs[:, kp, :, tc_i * 512:(tc_i + 1) * 512],
                            start=(i == 0), stop=(i == n - 1),
                            perf_mode=DR,
                        )
                        i += 1
                return acc, dst, bias

            def evict_qk_act(acc, dst, bias, hp, tc_i):
                nc.scalar.activation(
                    dst[:, hp, tc_i * 512:(tc_i + 1) * 512], acc[:],
                    ACT_F.Identity, bias=bias[:, hp:hp + 1], scale=1.0,
                )

            def evict_qk_dve(acc, dst, bias, hp, tc_i):
                nc.vector.tensor_scalar(
                    out=dst[:, hp, tc_i * 512:(tc_i + 1) * 512], in0=acc[:],
                    scalar1=bias[:, hp:hp + 1], scalar2=None, op0=ALU.add,
                )

            def qkv_v_unit(ts):
                """v token-major for token-subchunk ts (128 tokens), all 8
                heads: psum [128tok, 512ch] -> v_sb[:, ts, :, 0:64]."""
                acc = pp.tile([P, 512], F32, tag="acc", bufs=2,
                              name=f"acc_v_{ts}")
                terms = [(xh_sb, wt["wvh"]), (xl_sb, wt["wvh"]),
                         (xh_sb, wt["wvl"])]
                n = len(terms) * 4
                i = 0
                for xs, ws in terms:
                    for kp in range(4):
                        nc.tensor.matmul(
                            acc[:],
                            xs[:, kp, :, ts * P:(ts + 1) * P],
                            ws[:, kp, :, :],
                            start=(i == 0), stop=(i == n - 1),
                            perf_mode=DR,
                        )
                        i += 1
                nc.vector.tensor_tensor(
                    out=v_sb[:, ts, :, 0:64],
                    in0=acc[:].rearrange("p (h d) -> p h d", d=64),
                    in1=vbb[:].rearrange("p (h d) -> p h d", d=64),
                    op=ALU.add,
                )

            # ---- attention ----
            def attn_gen(hp, proj_ready):
                """Yields per chunk step (40 + 2 drain steps). vmm trails
                scores by 2 chunks so exp latency is hidden."""
                state = {}
                ready_delay = []

                def tick_ready():
                    for it in list(ready_delay):
                        n_, v_ = it
                        if n_ <= 0:
                            proj_ready.append(v_)
                            ready_delay.remove(it)
                        else:
                            ready_delay[ready_delay.index(it)] = (n_ - 1, v_)

                def emit_scores(jj, ck):
                    w_off = max(0, P * (ck - 4 * jj))
                    # widen the 128-wide diagonal matmul to 256 so the f32r
                    # moving operand stays at 1 cyc/row (ap >= 256); the
                    # extra columns land in psum that exp never reads
                    w_mm = min(w_off, 256)
                    diag = ck >= 4 * jj
                    sc = pp.tile([P, 1024], F32, tag="sc", bufs=2,
                                 name=f"sc_{hp}_{jj}_{ck}")
                    tqa = jj * 512
                    for l in range(2):
                        r0 = 64 * l
                        nc.tensor.matmul(
                            sc[:, 512 * l + w_mm: 512 * (l + 1)],
                            kT[r0:r0 + 64, hp, ck * P:(ck + 1) * P],
                            qT[r0:r0 + 64, hp, tqa + w_mm: tqa + 512],
                            start=True, stop=not diag,
                            tile_position=(r0, 0),
                        )
                    if diag:
                        for l in range(2):
                            nc.tensor.matmul(
                                sc[:, 512 * l + w_off: 512 * l + w_off + P],
                                identb[:], maskb[:],
                                start=False, stop=True,
                            )
                    ex = pe.tile([P, 1024], BF16, tag="ex", bufs=4)
                    if w_off == 0:
                        nc.scalar.activation(
                            ex[:], sc[:], ACT_F.Exp, scale=EXP_SCALE)
                    else:
                        # per-head exp: [512l, 512l+w_off) is unwritten psum
                        for l in range(2):
                            nc.scalar.activation(
                                ex[:, 512 * l + w_off: 512 * (l + 1)],
                                sc[:, 512 * l + w_off: 512 * (l + 1)],
                                ACT_F.Exp, scale=EXP_SCALE)
                    return (jj, ck, w_off, ex)

                def emit_vmm(pend):
                    jj, ck, w_off, ex = pend
                    nchunks = 4 * jj + 4
                    if ck == 0:
                        state[jj] = [
                            pp.tile([65, 512], F32, tag=f"po{l}", bufs=1,
                                    name=f"po_{hp}_{l}_{jj}")
                            for l in range(2)
                        ]
                    po = state[jj]
                    for l in range(2):
                        nc.tensor.matmul(
                            po[l][:, w_off:512],
                            v_sb[:, ck, 2 * hp + l, :],
                            ex[:, 512 * l + w_off: 512 * (l + 1)],
                            start=(ck == 0), stop=(ck == nchunks - 1),
                        )
                    if ck == nchunks - 1:
                        tqa = jj * 512
                        for l in range(2):
                            r0 = 64 * l
                            rc = psm.tile([1, 512], F32, tag="rc", bufs=1,
                                          name=f"rc_{hp}_{l}_{jj}")
                            nc.vector.reciprocal(rc[:], po[l][64:65, :])
                            # per-head staging at partition base 0: SBUF
                            # tensor_tensor inputs must share base partition
                            # (only the output may be partition-offset), and
                            # gpsimd broadcast must target base 0
                            un = psm.tile([64, 512], F32, tag="un", bufs=2,
                                          name=f"un_{hp}_{l}_{jj}")
                            nc.vector.tensor_copy(un[:], po[l][0:64, :])
                            rb = psm.tile([64, 512], F32, tag="rb", bufs=2,
                                          name=f"rb_{hp}_{l}_{jj}")
                            nc.gpsimd.partition_broadcast(rb[:], rc[0:1, :])
                            nc.vector.tensor_tensor(
                                out=attn[r0:r0 + 64, hp, tqa:tqa + 512],
                                in0=un[:], in1=rb[:],
                                op=ALU.mult,
                            )
                        kpr, sr = divmod(hp, 2)
                        nc.vector.tensor_copy(
                            attn8h[:, kpr, sr, tqa:tqa + 512],
                            attn[:, hp, tqa:tqa + 512],
                        )
                        nc.vector.tensor_tensor(
                            out=attn8l[:, kpr, sr, tqa:tqa + 512],
                            in0=attn[:, hp, tqa:tqa + 512],
                            in1=attn8h[:, kpr, sr, tqa:tqa + 512],
                            op=ALU.subtract,
                        )
                        del state[jj]
                        ready_delay.append((3, jj))

                seq = [(jj, ck) for jj in range(4) for ck in range(4 * jj + 4)]
                pend = []
                for (jj, ck) in seq:
                    pend.append(emit_scores(jj, ck))
                    if len(pend) > 2:
                        emit_vmm(pend.pop(0))
                    tick_ready()
                    yield
                while pend:
                    emit_vmm(pend.pop(0))
                    tick_ready()
                    yield
                for _ in range(6):
                    tick_ready()
                    yield

            # ---- proj ----
            def proj_unit(ob, jj):
                pj = pp.tile([P, 512], F32, tag="acc", bufs=2,
                             name=f"pj_{ob}_{jj}")
                terms = [(attn8h, pw8h), (attn8l, pw8h), (attn8h, pw8l)]
                n = len(terms) * 2
                i = 0
                for a8, w8 in terms:
                    for kp in range(2):
                        nc.tensor.matmul(
                            pj[:],
                            w8[:, kp, :, ob * P:(ob + 1) * P],
                            a8[:, kp, :, jj * 512:(jj + 1) * 512],
                            start=(i == 0), stop=(i == n - 1),
                            perf_mode=DR,
                        )
                        i += 1
                ysb = py.tile([P, 512], F32, tag="ysb",
                              name=f"ysb_{ob}_{jj}")
                nc.vector.tensor_scalar(
                    out=ysb[:], in0=pj[:], scalar1=1.0 / SQK, scalar2=None,
                    op0=ALU.mult)
                nc.sync.dma_start(
                    yT_d[ob * P:(ob + 1) * P, jj * 512:(jj + 1) * 512],
                    ysb[:],
                )

            def proj_gen(proj_ready):
                done = 0
                while done < 32:
                    if not proj_ready:
                        yield False
                        continue
                    jj = proj_ready.pop(0)
                    for ob in range(8):
                        proj_unit(ob, jj)
                        done += 1
                        yield True

            def drain(gen):
                if gen is None:
                    return
                for _ in gen:
                    pass

            # ---- phase A ----
            # All weight DMAs are queued before the first q/k unit (program
            # order defines the dependence graph - a tile read before its
            # DMA reads garbage). v units lead; attention chunks of jj
            # interleave into the v units of tc jj+1 once q/k(jj) landed.
            load_x(2, ntc=2)
            proj_ready = []
            at = attn_gen(0, [])
            for ts in range(4):
                qkv_v_unit(ts)
            for tc_i in range(4):
                for which in ("q", "k"):
                    acc, dst, bias = qkv_qk_unit(which, 0, tc_i)
                    evict_qk_act(acc, dst, bias, 0, tc_i)
                if tc_i + 1 < 4:
                    n_at = 4 * tc_i + 4  # chunks of jj = tc_i
                    for u in range(4):
                        qkv_v_unit(4 * (tc_i + 1) + u)
                        for _ in range((n_at + 3 - u) // (4 - u)):
                            next(at, None)
                            n_at -= 1
            ensure_pw()

            # ---- attention stages ----
            # filler units run at fixed steps (jj transitions) of each stage;
            # hp3 keeps its own tc3 q/k units back to fill its stage start,
            # where proj(jj0) is not yet released
            from collections import deque
            filler = deque()
            pr = None
            for hp in range(NHP):
                if hp > 0:
                    at = attn_gen(
                        hp, proj_ready if hp == NHP - 1 else [])
                if hp + 1 < NHP:
                    last = 3 if hp + 1 == NHP - 1 else 4
                    for tc_i in range(last):
                        for which in ("q", "k"):
                            filler.append((which, hp + 1, tc_i))
                if hp == NHP - 1:
                    for which in ("q", "k"):
                        filler.append((which, hp, 3))
                    pr = proj_gen(proj_ready)
                fill_at = {0, 1, 4, 5, 12, 13, 14, 24, 25, 26}
                i = 0
                for _ in at:
                    if filler and i in fill_at:
                        which, fhp, ftc = filler.popleft()
                        acc, dst, bias = qkv_qk_unit(which, fhp, ftc)
                        evict_qk_dve(acc, dst, bias, fhp, ftc)
                    if pr is not None:
                        next(pr, None)
                    i += 1
                while filler and hp + 1 < NHP:
                    which, fhp, ftc = filler.popleft()
                    acc, dst, bias = qkv_qk_unit(which, fhp, ftc)
                    evict_qk_dve(acc, dst, bias, fhp, ftc)
            drain(pr)

            if DBG:
                nc.sync.dma_start(qT_dbg[:], qT[:].bitcast(F32).rearrange("p a b -> p (a b)"))
                nc.sync.dma_start(kT_dbg[:], kT[:].bitcast(F32).rearrange("p a b -> p (a b)"))
                nc.sync.dma_start(at_dbg[:], attn[:].rearrange("p a b -> p (a b)"))
                nc.sync.dma_start(v_dbg[:], v_sb[:].rearrange("p a b c -> p (a b c)"))
                nc.sync.dma_start(a8h_dbg[:], attn8h[:].rearrange("p a b c -> p (a b c)"))
                nc.sync.dma_start(a8l_dbg[:], attn8l[:].rearrange("p a b c -> p (a b c)"))

    nc.compile()
    return nc


def get_nc():
    if "nc" not in _CACHED:
        _CACHED["nc"] = _build()
    return _CACHED["nc"]


def make_in_maps(x, qkv_w, qkv_b, proj_w):
    x = np.asarray(x, dtype=np.float32)
    qkv_w = np.asarray(qkv_w, dtype=np.float32)
    qkv_b = np.asarray(qkv_b, dtype=np.float32)
    proj_w = np.asarray(proj_w, dtype=np.float32)

    def fp8_split(a):
        hi = a.astype(E4M3)
        lo = (a - hi.astype(np.float32)).astype(E4M3)
        return hi, lo

    xs = []
    for b in range(B):
        xT = np.ascontiguousarray(x[b].T) * np.float32(SX)
        xs.append(fp8_split(xT))

    in_maps = []
    for c in range(NCORES):
        b, gidx = divmod(c, 2)
        cs = slice(GC * gidx, GC * (gidx + 1))
        xh, xl = xs[b]

        def wsplit(wslice):
            wT = np.ascontiguousarray(wslice.T) * np.float32(SW)
            return fp8_split(wT)

        wqh, wql = wsplit(qkv_w[cs, :])
        wkh, wkl = wsplit(qkv_w[C:][cs, :])
        wvh, wvl = wsplit(qkv_w[2 * C:][cs, :])
        pwh, pwl = fp8_split(
            np.ascontiguousarray(proj_w[:, cs].T) * np.float32(SW))
        in_maps.append({
            "xh": xh, "xl": xl,
            "wqh": wqh, "wql": wql, "wkh": wkh, "wkl": wkl,
            "wvh": wvh, "wvl": wvl,
            "pwh": pwh, "pwl": pwl,
            "qb64": np.ascontiguousarray(
                (qkv_b[cs] * SQK).reshape(NHP, P).T.astype(np.float32)),
            "kb64": np.ascontiguousarray(
                (qkv_b[C:][cs] * SQK).reshape(NHP, P).T.astype(np.float32)),
            "vb64": (qkv_b[2 * C:][cs] * SQK).reshape(1, GC).astype(np.float32),
        })
    return in_maps


def kernel(x, qkv_w, qkv_b, proj_w, proj_b):
    proj_b = np.asarray(proj_b, dtype=np.float32)
    in_maps = make_in_maps(x, qkv_w, qkv_b, proj_w)
    nc = get_nc()
    res = run_bass_kernel_spmd(nc, in_maps, list(range(NCORES)))

    y = np.empty((B, T, C), dtype=np.float32)
    pb = proj_b.astype(np.float64)[None, :]
    for b in range(B):
        yT = res.results[2 * b]["yT"].astype(np.float64)
        yT += res.results[2 * b + 1]["yT"]
        y[b] = (yT.T + pb).astype(np.float32)
    return y


# revision 36
# speedup vs baseline: 1.0146x; 1.0146x over previous
"""Causal self-attention for TRN2, 8 NeuronCores.

Problem (hardcoded): B=4, T=2048, C=1024, H=16 heads, hd=64.
  qkv = x @ qkv_w.T + qkv_b ; per-head causal softmax(q k^T / 8) @ v ; out @ proj_w.T + proj_b

Sharding: core c -> (batch b = c//2, head-half g = c%2). Each core owns one
batch and 8 heads (512 q/k/v channels), so x traffic is 1/4 of the
all-batches variant and the proj partial is [1024, 2048] summed host-side
over just 2 cores per batch.

qkv projection runs in compensated fp8 (e4m3) with DoubleRow matmuls:
host pre-splits x and 64*w into (hi, lo) fp8 pairs and the device computes
xh@wh + xh@wl + xl@wh, each a 2-subtile (256-contraction) DR matmul at
0.5 cycles/row -- 3/4 the PE cycles of fp32r at ~1e-3 total error.
All scale factors are absorbed algebraically: q,k,v live as 64*(q,k,v) in
bf16, the exp scale is 0.125/64^2 = 2^-15, and the softmax-sum ones column
is 64.0 so the v scale cancels in the reciprocal-normalize.

v is produced directly token-major (stationary = x chunk, moving = wv), so
no PE transposes are needed for the attention v-matmul.

Attention per head-pair hp (2 heads packed on 128 partitions):
  scoresT[kt, q] per 128-token k-chunk via bf16 matmuls (+ additive -1e9
  causal mask on the diagonal 128 block), exp via ACT (psum -> bf16 sbuf),
  poT[d, q] = [64*v | 64].T @ ex accumulated in PSUM (row 64 = 64*softmax
  sums), normalization via DVE reciprocal + gpsimd partition-broadcast +
  DVE multiply into a bf16 attn slab.

proj: yT[o, t] = pw.T @ attn contracting this core's 512 channels over 4
slab matmuls; Pool evicts psum -> sbuf; host sums 2 partials per batch.

Engine budget per core (cost model): PE ~508K cycles (~212us), ACT ~157us
(exp), DVE ~60us, Pool ~90us, DMA ~17MB (~50us). Software pipelining:
phase A does qkv for v(all heads) + q/k(hp0); each attention stage hp
interleaves q/k units of hp+1; hp3 interleaves proj units per finished
512-token block.
"""

import numpy as np
import ml_dtypes

import concourse.mybir as mybir
import concourse.tile as tile
from concourse import bacc
from concourse.bass_utils import run_bass_kernel_spmd
from concourse.masks import make_identity

F32 = mybir.dt.float32
F32R = mybir.dt.float32r
BF16 = mybir.dt.bfloat16
F8 = mybir.dt.float8e4
ACT_F = mybir.ActivationFunctionType
ALU = mybir.AluOpType
DR = mybir.MatmulPerfMode.DoubleRow
E4M3 = ml_dtypes.float8_e4m3

B, T, C, H, HD = 4, 2048, 1024, 16, 64
NCORES = 8
P = 128
GC = 512            # channels per core (8 heads)
NHP = 4             # head-pairs per core
NEG = -1.0e12
SX = 16.0           # host-side x prescale (keeps fp8 lo-terms normal-range)
SW = 512.0          # host-side weight prescale
SQK = SX * SW       # scale carried by q,k,v in sbuf (8192)
ONES = SW           # softmax-sum column: denom scale 512 -> attn slab = 16*attn
EXP_SCALE = 0.125 / (SQK * SQK)   # exact 2^-29

_CACHED = {}


def _round_fp32r(a: np.ndarray) -> np.ndarray:
    u = np.ascontiguousarray(a, dtype=np.float32).view(np.uint32)
    lsb = (u >> 12) & 1
    out = ((u + 0x7FF + lsb) & np.uint32(0xFFFFF000)).view(np.float32)
    return np.where(np.isfinite(a), out, a).astype(np.float32)


class _Ctx:
    pass


def _build():
    nc = bacc.Bacc("TRN2", target_bir_lowering=False, debug=False)

    xh_d = nc.dram_tensor("xh", [C, T], F8, kind="ExternalInput").ap()
    xl_d = nc.dram_tensor("xl", [C, T], F8, kind="ExternalInput").ap()
    w_ds = {}
    for name in ("wqh", "wql", "wkh", "wkl", "wvh", "wvl"):
        w_ds[name] = nc.dram_tensor(name, [C, GC], F8, kind="ExternalInput").ap()
    pwh_d = nc.dram_tensor("pwh", [GC, C], F8, kind="ExternalInput").ap()
    pwl_d = nc.dram_tensor("pwl", [GC, C], F8, kind="ExternalInput").ap()
    qb_d = nc.dram_tensor("qb64", [P, NHP], F32, kind="ExternalInput").ap()
    kb_d = nc.dram_tensor("kb64", [P, NHP], F32, kind="ExternalInput").ap()
    vb_d = nc.dram_tensor("vb64", [1, GC], F32, kind="ExternalInput").ap()
    yT_d = nc.dram_tensor("yT", [C, T], BF16, kind="ExternalOutput").ap()
    import os
    DBG = os.environ.get("KDBG", "0") == "1"
    if DBG:
        qT_dbg = nc.dram_tensor("qT_dbg", [P, NHP * T], F32, kind="ExternalOutput").ap()
        kT_dbg = nc.dram_tensor("kT_dbg", [P, NHP * T], F32, kind="ExternalOutput").ap()
        v_dbg = nc.dram_tensor("v_dbg", [P, 16 * 8 * 65], BF16, kind="ExternalOutput").ap()
        at_dbg = nc.dram_tensor("at_dbg", [P, NHP * T], BF16, kind="ExternalOutput").ap()
        a8h_dbg = nc.dram_tensor("a8h_dbg", [P, NHP * T], F8, kind="ExternalOutput").ap()
        a8l_dbg = nc.dram_tensor("a8l_dbg", [P, NHP * T], F8, kind="ExternalOutput").ap()

    with tile.TileContext(nc) as tc:
        with (
            tc.tile_pool(name="const", bufs=1) as pc,
            tc.tile_pool(name="exp", bufs=4) as pe,
            tc.tile_pool(name="small", bufs=2) as psm,
            tc.tile_pool(name="yev", bufs=3) as py,
            tc.tile_pool(name="ps", bufs=1, space="PSUM") as pp,
        ):
            g = _Ctx()
            g.nc = nc

            # PE warmup: all-zero matmuls bridge the initial DMA latency so
            # the tensor engine enters the real stream already ramped (the
            # cost model halves matmul speed until 3us of continuous busy)
            zed = pc.tile([P, 512], BF16, tag="zed")
            nc.vector.memset(zed[:], 0.0)
            for wi in range(20):
                dps = pp.tile([P, 512], F32, tag="acc", bufs=2,
                              name=f"warm_{wi}")
                nc.tensor.matmul(dps[:], zed[:, 0:P], zed[:],
                                 start=True, stop=True)

            # ---- constants ----
            identb = pc.tile([P, P], BF16, tag="identb")
            make_identity(nc, identb)
            maskb = pc.tile([P, P], BF16, tag="maskb")
            nc.gpsimd.memset(maskb[:], 0.0)
            nc.gpsimd.affine_select(
                out=maskb[:], in_=maskb[:],
                compare_op=ALU.is_ge, fill=NEG, base=0,
                pattern=[[1, P]], channel_multiplier=-1,
            )

            # ---- weights / biases ----
            # first-compute-critical DMAs first, split per-kp so the first
            # matmul only waits for one kp slice of wvh + x0h
            wt = {}
            for name in ("wvh", "wvl", "wqh", "wql", "wkh", "wkl"):
                t = pc.tile([P, 4, 2, GC], F8, tag=name, name=f"w_{name}")
                wt[name] = t
            xh_sb = pc.tile([P, 4, 2, T], F8, tag="xh")
            xl_sb = pc.tile([P, 4, 2, T], F8, tag="xl")

            def load_x(tc_i, hl="hl", ntc=1):
                pairs = {"h": (xh_sb, xh_d), "l": (xl_sb, xl_d)}
                for key in hl:
                    sb, d = pairs[key]
                    nc.sync.dma_start(
                        sb[:, :, :, tc_i * 512:(tc_i + ntc) * 512],
                        d.rearrange("(kp s p) t -> p kp s t", p=P, s=2)
                        [:, :, :, tc_i * 512:(tc_i + ntc) * 512],
                    )

            def load_w(name):
                nc.sync.dma_start(
                    wt[name][:],
                    w_ds[name].rearrange("(kp s p) c -> p kp s c", p=P, s=2),
                )

            # DMA queue ordered by first use, whole-tensor transfers (the
            # HWDGE descriptor-gen costs ~625ns per DMA, so fine splitting
            # paces PE at the HWDGE rate instead of the transfer rate)
            load_w("wvh")
            load_x(0, "h")
            vbrow = pc.tile([1, GC], F32, tag="vbrow")
            nc.sync.dma_start(vbrow[:], vb_d[:])
            load_x(0, "l")
            load_w("wvl")
            qb = pc.tile([P, NHP], F32, tag="qb")
            nc.sync.dma_start(qb[:], qb_d[:])
            kb = pc.tile([P, NHP], F32, tag="kb")
            nc.sync.dma_start(kb[:], kb_d[:])
            load_w("wqh")
            load_w("wql")
            load_w("wkh")
            load_w("wkl")
            load_x(1)
            vbb = pc.tile([P, GC], F32, tag="vbb")
            nc.gpsimd.partition_broadcast(vbb[:], vbrow[0:1, :])

            pw8h = pc.tile([P, 2, 2, C], F8, tag="pw8h")
            pw8l = pc.tile([P, 2, 2, C], F8, tag="pw8l")
            g.pw_loaded = False

            def ensure_pw():
                if not g.pw_loaded:
                    for t, d in ((pw8h, pwh_d), (pw8l, pwl_d)):
                        nc.sync.dma_start(
                            t[:],
                            d.rearrange("(kp s p) o -> p kp s o", p=P, s=2))
                    g.pw_loaded = True

            # ---- big state ----
            # qT/kT/attn are F32R: f32-family moving operands avoid the
            # per-matmul InstLdweights (71ns PE SEQ each) that non-f32
            # ifmaps require; ex/v_sb stay BF16 (SEQ has headroom there).
            qT = pc.tile([P, NHP, T], F32R, tag="qT")     # 64*q, part = l*64+d
            kT = pc.tile([P, NHP, T], F32R, tag="kT")     # 64*k
            v_sb = pc.tile([P, 16, 8, 65], BF16, tag="v_sb")  # [tok, ck, h, d|one]
            attn = pc.tile([P, NHP, T], BF16, tag="attn")
            attn8h = pc.tile([P, 2, 2, T], F8, tag="attn8h")
            attn8l = pc.tile([P, 2, 2, T], F8, tag="attn8l")
            # softmax-sum column: 64.0 so the 64*v scale cancels at normalize
            nc.vector.memset(v_sb[:, :, :, 64:65], ONES)

            # ---- qkv units (fp8 DoubleRow, 3-term compensated) ----
            def qkv_qk_unit(which, hp, tc_i):
                """q or k for head-pair hp over 512 tokens: psum [128ch, 512]."""
                wh, wl = ("wqh", "wql") if which == "q" else ("wkh", "wkl")
                dst, bias = (qT, qb) if which == "q" else (kT, kb)
                acc = pp.tile([P, 512], F32, tag="acc", bufs=2,
                              name=f"acc_{which}_{hp}_{tc_i}")
                terms = [(xh_sb, wt[wh]), (xl_sb, wt[wh]), (xh_sb, wt[wl])]
                n = len(terms) * 4
                i = 0
                for xs, ws in terms:
                    for kp in range(4):
                        nc.tensor.matmul(
                            acc[:],
                            ws[:, kp, :, hp * P:(hp + 1) * P],
                            xs[:, kp, :, tc_i * 512:(tc_i + 1) * 512],
                            start=(i == 0), stop=(i == n - 1),
                            perf_mode=DR,
                        )
                        i += 1
                return acc, dst, bias

            def evict_qk_act(acc, dst, bias, hp, tc_i):
                nc.scalar.activation(
                    dst[:, hp, tc_i * 512:(tc_i + 1) * 512], acc[:],
                    ACT_F.Identity, bias=bias[:, hp:hp + 1], scale=1.0,
                )

            def evict_qk_dve(acc, dst, bias, hp, tc_i):
                nc.vector.tensor_scalar(
                    out=dst[:, hp, tc_i * 512:(tc_i + 1) * 512], in0=acc[:],
                    scalar1=bias[:, hp:hp + 1], scalar2=None, op0=ALU.add,
                )

            def qkv_v_unit(ts):
                """v token-major for token-subchunk ts (128 tokens), all 8
                heads: psum [128tok, 512ch] -> v_sb[:, ts, :, 0:64]."""
                acc = pp.tile([P, 512], F32, tag="acc", bufs=2,
                              name=f"acc_v_{ts}")
                terms = [(xh_sb, wt["wvh"]), (xl_sb, wt["wvh"]),
                         (xh_sb, wt["wvl"])]
                n = len(terms) * 4
                i = 0
                for xs, ws in terms:
                    for kp in range(4):
                        nc.tensor.matmul(
                            acc[:],
                            xs[:, kp, :, ts * P:(ts + 1) * P],
                            ws[:, kp, :, :],
                            start=(i == 0), stop=(i == n - 1),
                            perf_mode=DR,
                        )
                        i += 1
                nc.vector.tensor_tensor(
                    out=v_sb[:, ts, :, 0:64],
                    in0=acc[:].rearrange("p (h d) -> p h d", d=64),
                    in1=vbb[:].rearrange("p (h d) -> p h d", d=64),
                    op=ALU.add,
                )

            # ---- attention ----
            def attn_gen(hp, proj_ready):
                """Yields per chunk step (40 + 2 drain steps). vmm trails
                scores by 2 chunks so exp latency is hidden."""
                state = {}
                ready_delay = []

                def tick_ready():
                    for it in list(ready_delay):
                        n_, v_ = it
                        if n_ <= 0:
                            proj_ready.append(v_)
                            ready_delay.remove(it)
                        else:
                            ready_delay[ready_delay.index(it)] = (n_ - 1, v_)

                def emit_scores(jj, ck):
                    w_off = max(0, P * (ck - 4 * jj))
                    # widen the 128-wide diagonal matmul to 256 so the f32r
                    # moving operand stays at 1 cyc/row (ap >= 256); the
                    # extra columns land in psum that exp never reads
                    w_mm = min(w_off, 256)
                    diag = ck >= 4 * jj
                    sc = pp.tile([P, 1024], F32, tag="sc", bufs=2,
                                 name=f"sc_{hp}_{jj}_{ck}")
                    tqa = jj * 512
                    for l in range(2):
                        r0 = 64 * l
                        nc.tensor.matmul(
                            sc[:, 512 * l + w_mm: 512 * (l + 1)],
                            kT[r0:r0 + 64, hp, ck * P:(ck + 1) * P],
                            qT[r0:r0 + 64, hp, tqa + w_mm: tqa + 512],
                            start=True, stop=not diag,
                            tile_position=(r0, 0),
                        )
                    if diag:
                        for l in range(2):
                            nc.tensor.matmul(
                                sc[:, 512 * l + w_off: 512 * l + w_off + P],
                                identb[:], maskb[:],
                                start=False, stop=True,
                            )
                    ex = pe.tile([P, 1024], BF16, tag="ex", bufs=4)
                    if w_off == 0:
                        nc.scalar.activation(
                            ex[:], sc[:], ACT_F.Exp, scale=EXP_SCALE)
                    else:
                        # per-head exp: [512l, 512l+w_off) is unwritten psum
                        for l in range(2):
                            nc.scalar.activation(
                                ex[:, 512 * l + w_off: 512 * (l + 1)],
                                sc[:, 512 * l + w_off: 512 * (l + 1)],
                                ACT_F.Exp, scale=EXP_SCALE)
                    return (jj, ck, w_off, ex)

                def emit_vmm(pend):
                    jj, ck, w_off, ex = pend
                    nchunks = 4 * jj + 4
                    if ck == 0:
                        state[jj] = [
                            pp.tile([65, 512], F32, tag=f"po{l}", bufs=1,
                                    name=f"po_{hp}_{l}_{jj}")
                            for l in range(2)
                        ]
                    po = state[jj]
                    for l in range(2):
                        nc.tensor.matmul(
                            po[l][:, w_off:512],
                            v_sb[:, ck, 2 * hp + l, :],
                            ex[:, 512 * l + w_off: 512 * (l + 1)],
                            start=(ck == 0), stop=(ck == nchunks - 1),
                        )
                    if ck == nchunks - 1:
                        tqa = jj * 512
                        # per-head staging at partition base 0: SBUF
                        # tensor_tensor inputs must share base partition
                        # (only the output may be partition-offset), and
                        # gpsimd broadcast must target base 0. The psum
                        # readers (rc, un) for both heads run first so the
                        # po banks free before the broadcast/multiply tail.
                        rcs, uns = [], []
                        for l in range(2):
                            rc = psm.tile([1, 512], F32, tag="rc", bufs=2,
                                          name=f"rc_{hp}_{l}_{jj}")
                            nc.vector.reciprocal(rc[:], po[l][64:65, :])
                            un = psm.tile([64, 512], F32, tag="un", bufs=2,
                                          name=f"un_{hp}_{l}_{jj}")
                            if hp == NHP - 1 and jj == 3:
                                # ACT is exp-idle by now; parallel with rc
                                nc.scalar.activation(
                                    un[:], po[l][0:64, :], ACT_F.Copy)
                            else:
                                nc.vector.tensor_copy(un[:], po[l][0:64, :])
                            rcs.append(rc)
                            uns.append(un)
                        for l in range(2):
                            r0 = 64 * l
                            rb = psm.tile([64, 512], F32, tag="rb", bufs=2,
                                          name=f"rb_{hp}_{l}_{jj}")
                            nc.gpsimd.partition_broadcast(rb[:], rcs[l][0:1, :])
                            eng = nc.vector
                            if hp == NHP - 1 and jj == 3 and l == 0:
                                eng = nc.gpsimd
                            eng.tensor_tensor(
                                out=attn[r0:r0 + 64, hp, tqa:tqa + 512],
                                in0=uns[l][:], in1=rb[:],
                                op=ALU.mult,
                            )
                        kpr, sr = divmod(hp, 2)
                        nc.vector.tensor_copy(
                            attn8h[:, kpr, sr, tqa:tqa + 512],
                            attn[:, hp, tqa:tqa + 512],
                        )
                        sub_eng = (nc.vector if hp == NHP - 1 and jj == 3
                                   else nc.gpsimd)
                        sub_eng.tensor_tensor(
                            out=attn8l[:, kpr, sr, tqa:tqa + 512],
                            in0=attn[:, hp, tqa:tqa + 512],
                            in1=attn8h[:, kpr, sr, tqa:tqa + 512],
                            op=ALU.subtract,
                        )
                        del state[jj]
                        ready_delay.append((3, jj))

                seq = [(jj, ck) for jj in range(4) for ck in range(4 * jj + 4)]
                pend = []
                for (jj, ck) in seq:
                    pend.append(emit_scores(jj, ck))
                    if len(pend) > 2:
                        emit_vmm(pend.pop(0))
                    tick_ready()
                    yield
                while pend:
                    emit_vmm(pend.pop(0))
                    tick_ready()
                    yield
                for _ in range(6):
                    tick_ready()
                    yield

            # ---- proj ----
            def proj_unit(ob, jj, tail=False):
                pj = pp.tile([P, 512], F32, tag="acc", bufs=2,
                             name=f"pj_{ob}_{jj}")
                terms = [(attn8h, pw8h), (attn8l, pw8h), (attn8h, pw8l)]
                n = len(terms) * 2
                i = 0
                for kp in range(2):
                    for a8, w8 in terms:
                        nc.tensor.matmul(
                            pj[:],
                            w8[:, kp, :, ob * P:(ob + 1) * P],
                            a8[:, kp, :, jj * 512:(jj + 1) * 512],
                            start=(i == 0), stop=(i == n - 1),
                            perf_mode=DR,
                        )
                        i += 1
                ysb = py.tile([P, 512], BF16, tag="ysb",
                              name=f"ysb_{ob}_{jj}")
                if tail and ob % 2 == 0:
                    nc.scalar.activation(
                        ysb[:], pj[:], ACT_F.Copy, scale=1.0 / SQK)
                else:
                    nc.vector.tensor_scalar(
                        out=ysb[:], in0=pj[:], scalar1=1.0 / SQK,
                        scalar2=None, op0=ALU.mult)
                nc.sync.dma_start(
                    yT_d[ob * P:(ob + 1) * P, jj * 512:(jj + 1) * 512],
                    ysb[:],
                )

            def proj_gen(proj_ready):
                done = 0
                while done < 32:
                    if not proj_ready:
                        yield False
                        continue
                    jj = proj_ready.pop(0)
                    for ob in range(8):
                        proj_unit(ob, jj, tail=(done >= 24))
                        done += 1
                        yield True

            def drain(gen):
                if gen is None:
                    return
                for _ in gen:
                    pass

            # ---- phase A ----
            # All weight DMAs are queued before the first q/k unit (program
            # order defines the dependence graph - a tile read before its
            # DMA reads garbage). v units lead; attention chunks of jj
            # interleave into the v units of tc jj+1 once q/k(jj) landed.
            load_x(2, ntc=2)
            proj_ready = []
            at = attn_gen(0, [])
            for ts in range(4):
                qkv_v_unit(ts)
            for tc_i in range(4):
                for which in ("q", "k"):
                    acc, dst, bias = qkv_qk_unit(which, 0, tc_i)
                    evict_qk_act(acc, dst, bias, 0, tc_i)
                if tc_i + 1 < 4:
                    n_at = 4 * tc_i + 4  # chunks of jj = tc_i
                    for u in range(4):
                        qkv_v_unit(4 * (tc_i + 1) + u)
                        for _ in range((n_at + 3 - u) // (4 - u)):
                            next(at, None)
                            n_at -= 1
            ensure_pw()

            # ---- attention stages ----
            # filler units run at fixed steps (jj transitions) of each stage;
            # hp3 keeps its own tc3 q/k units back to fill its stage start,
            # where proj(jj0) is not yet released
            from collections import deque
            filler = deque()
            pr = None
            for hp in range(NHP):
                if hp > 0:
                    at = attn_gen(
                        hp, proj_ready if hp == NHP - 1 else [])
                if hp + 1 < NHP:
                    last = 2 if hp + 1 == NHP - 1 else 4
                    for tc_i in range(last):
                        for which in ("q", "k"):
                            filler.append((which, hp + 1, tc_i))
                if hp == NHP - 1:
                    for tc_i in (2, 3):
                        for which in ("q", "k"):
                            filler.append((which, hp, tc_i))
                    pr = proj_gen(proj_ready)
                nf = max(1, len(filler))
                fill_at = {0, 1} | {round(k * 44 / nf) + 2 for k in range(nf)}
                i = 0
                for _ in at:
                    if filler and i in fill_at:
                        which, fhp, ftc = filler.popleft()
                        acc, dst, bias = qkv_qk_unit(which, fhp, ftc)
                        evict_qk_dve(acc, dst, bias, fhp, ftc)
                    if pr is not None:
                        next(pr, None)
                    i += 1
                while filler and hp + 1 < NHP:
                    which, fhp, ftc = filler.popleft()
                    acc, dst, bias = qkv_qk_unit(which, fhp, ftc)
                    evict_qk_dve(acc, dst, bias, fhp, ftc)
            drain(pr)

            if DBG:
                nc.sync.dma_start(qT_dbg[:], qT[:].bitcast(F32).rearrange("p a b -> p (a b)"))
                nc.sync.dma_start(kT_dbg[:], kT[:].bitcast(F32).rearrange("p a b -> p (a b)"))
                nc.sync.dma_start(at_dbg[:], attn[:].rearrange("p a b -> p (a b)"))
                nc.sync.dma_start(v_dbg[:], v_sb[:].rearrange("p a b c -> p (a b c)"))
                nc.sync.dma_start(a8h_dbg[:], attn8h[:].rearrange("p a b c -> p (a b c)"))
                nc.sync.dma_start(a8l_dbg[:], attn8l[:].rearrange("p a b c -> p (a b c)"))

    nc.compile()
    return nc


def get_nc():
    if "nc" not in _CACHED:
        _CACHED["nc"] = _build()
    return _CACHED["nc"]


def make_in_maps(x, qkv_w, qkv_b, proj_w):
    x = np.asarray(x, dtype=np.float32)
    qkv_w = np.asarray(qkv_w, dtype=np.float32)
    qkv_b = np.asarray(qkv_b, dtype=np.float32)
    proj_w = np.asarray(proj_w, dtype=np.float32)

    def fp8_split(a):
        hi = a.astype(E4M3)
        lo = (a - hi.astype(np.float32)).astype(E4M3)
        return hi, lo

    xs = []
    for b in range(B):
        xT = np.ascontiguousarray(x[b].T) * np.float32(SX)
        xs.append(fp8_split(xT))

    in_maps = []
    for c in range(NCORES):
        b, gidx = divmod(c, 2)
        cs = slice(GC * gidx, GC * (gidx + 1))
        xh, xl = xs[b]

        def wsplit(wslice):
            wT = np.ascontiguousarray(wslice.T) * np.float32(SW)
            return fp8_split(wT)

        wqh, wql = wsplit(qkv_w[cs, :])
        wkh, wkl = wsplit(qkv_w[C:][cs, :])
        wvh, wvl = wsplit(qkv_w[2 * C:][cs, :])
        pwh, pwl = fp8_split(
            np.ascontiguousarray(proj_w[:, cs].T) * np.float32(SW))
        in_maps.append({
            "xh": xh, "xl": xl,
            "wqh": wqh, "wql": wql, "wkh": wkh, "wkl": wkl,
            "wvh": wvh, "wvl": wvl,
            "pwh": pwh, "pwl": pwl,
            "qb64": np.ascontiguousarray(
                (qkv_b[cs] * SQK).reshape(NHP, P).T.astype(np.float32)),
            "kb64": np.ascontiguousarray(
                (qkv_b[C:][cs] * SQK).reshape(NHP, P).T.astype(np.float32)),
            "vb64": (qkv_b[2 * C:][cs] * SQK).reshape(1, GC).astype(np.float32),
        })
    return in_maps


def kernel(x, qkv_w, qkv_b, proj_w, proj_b):
    proj_b = np.asarray(proj_b, dtype=np.float32)
    in_maps = make_in_maps(x, qkv_w, qkv_b, proj_w)
    nc = get_nc()
    res = run_bass_kernel_spmd(nc, in_maps, list(range(NCORES)))

    y = np.empty((B, T, C), dtype=np.float32)
    pb = proj_b.astype(np.float64)[None, :]
    for b in range(B):
        yT = res.results[2 * b]["yT"].astype(np.float64)
        yT += res.results[2 * b + 1]["yT"].astype(np.float64)
        y[b] = (yT.T + pb).astype(np.float32)
    return y
